# revision 8
# baseline (speedup 1.0000x reference)
"""Triangle-symmetric contrastive loss on 8 TRN2 cores.

Exploits sim-matrix symmetry: each unordered 128x1024 block-pair of the
similarity matrix is computed once. Rows are label-sorted on the host.
Work items per core (identical SPMD stream, per-core DATA differs):
  - 8 diagonal tiles: (I-block 8c+u, J-super c), full width, BIG on diag.
  - 28 off-diagonal tiles in 7 groups of 4; each group shares one J-super
    g > I-super. Row side: exp + split accum (row sums). Column side
    (mirror): matmul with stationary [ones | labels_of_I] over the bf16
    exp tile accumulates [all_col_sum; pos_col_sum] in PSUM across the
    group's 4 tiles.
Device outputs raw accumulators; the host epilogue assembles all/pos sums
per row, applies the log-loss, and averages. Total multiplied elements:
~N^2/2 + N*1024/2 vs N^2 for the full-matrix kernel.
"""

import numpy as np

import concourse.bass as bass
import concourse.tile as tile
import concourse.mybir as mybir
from concourse import bacc
from concourse.bass_utils import run_bass_kernel_spmd

N, D = 8192, 1024
NCORES = 8
P = 128
JW = 1024            # J-super width
NSUP = N // JW       # 8 J-supers
NBLK = N // P        # 64 I-blocks
ND = D // P          # 8 contraction chunks
NDIAG = NSUP         # diag tiles per core (8)
NGRP = 7             # off-diag groups per core
GSZ = 4              # tiles per group
NOFF = NGRP * GSZ    # 28
NT = NDIAG + NOFF    # 36 tiles per core
SCALE = 10.0
EPS = 1e-8
BIG = 1e9

F32 = mybir.dt.float32
BF16 = mybir.dt.bfloat16
DT_MM = mybir.dt.float8e4

_build_cache = {}


def plan(n0: int):
    """Returns (b, per-core iteration plans).

    plan[c] = dict(tiles=[(t, g) x 36], groups=[g x 7])
    """
    b = n0 % JW
    # off-diag (t, g) pairs grouped: for g, pairs t in 0..8g-1, chunked by 4
    groups = []  # (g, [t0..t3])
    for g in range(1, NSUP):
        ts = list(range(8 * g))
        for k in range(0, len(ts), GSZ):
            groups.append((g, ts[k : k + GSZ]))
    assert len(groups) == NCORES * NGRP
    plans = []
    for c in range(NCORES):
        gsel = [groups[c + NCORES * q] for q in range(NGRP)]
        tiles = [(8 * c + u, c) for u in range(NDIAG)]
        for g, ts in gsel:
            tiles += [(t, g) for t in ts]
        plans.append({"tiles": tiles, "groups": [g for g, _ in gsel]})
    return b, plans


def build(reps: int = 1, b: int = 512):
    key = (reps, b)
    if key in _build_cache:
        return _build_cache[key]
    ns = 2 if b > 0 else 1
    NW = NT * ns

    nc = bacc.Bacc("TRN2", target_bir_lowering=False, debug=False)
    # stationary blocks packed in iteration order: [P, ND, NT*P]
    stat_d = nc.dram_tensor("statp", [D, NT * P], DT_MM, kind="ExternalInput")
    # moving supers packed in use order (diag super + 7 group supers)
    mov_d = nc.dram_tensor("movp", [D, (1 + NGRP) * JW], DT_MM, kind="ExternalInput")
    # per-tile [ones | labels of I-block] stationary for mirror matmuls
    colw_d = nc.dram_tensor("colw", [P, NT, 2], BF16, kind="ExternalInput")
    rowacc_d = nc.dram_tensor("rowacc", [P, NW], F32, kind="ExternalOutput")
    mirror_d = nc.dram_tensor("mirror", [2 * NGRP, JW], F32, kind="ExternalOutput")

    statp = stat_d.ap().rearrange("(dc p) n -> p dc n", p=P)
    movp = mov_d.ap().rearrange("(dc p) n -> p dc n", p=P)

    with tile.TileContext(nc) as tc:
        with (
            tc.tile_pool(name="consts", bufs=1) as consts,
            tc.tile_pool(name="movb", bufs=2) as movb,
            tc.tile_pool(name="expp", bufs=3) as expp,
            tc.tile_pool(name="stats", bufs=1) as stats,
            tc.tile_pool(name="fin", bufs=1) as fin,
            tc.tile_pool(name="psum", bufs=3, space=bass.MemorySpace.PSUM) as psum,
            tc.tile_pool(name="mpsum", bufs=1, space=bass.MemorySpace.PSUM) as mpsum,
        ):
            # stationary: diag part first so compute can start early
            statb = consts.tile([P, ND, NT * P], DT_MM)
            nc.sync.dma_start(
                out=statb[:, :, 0 : NDIAG * P], in_=statp[:, :, 0 : NDIAG * P]
            )
            for q in range(NGRP):
                lo = (NDIAG + q * GSZ) * P
                hi = (NDIAG + (q + 1) * GSZ) * P
                nc.sync.dma_start(
                    out=statb[:, :, lo:hi], in_=statp[:, :, lo:hi]
                )
            colw = consts.tile([P, NT, 2], BF16)
            nc.sync.dma_start(out=colw, in_=colw_d.ap())
            bigI = consts.tile([P, P], F32)
            nc.gpsimd.memset(bigI, 0.0)
            nc.gpsimd.affine_select(
                out=bigI,
                in_=bigI,
                compare_op=mybir.AluOpType.not_equal,
                fill=BIG,
                base=0,
                pattern=[[-1, P]],
                channel_multiplier=1,
            )

            for rep in range(reps):
                rowacc = stats.tile([P, NW], F32, tag="rowacc")
                mir = None

                for m in range(NT):
                    is_diag = m < NDIAG
                    # moving buffer: one DMA per diag-block / group
                    if m == 0 or (not is_diag and (m - NDIAG) % GSZ == 0):
                        mv_idx = 0 if is_diag else 1 + (m - NDIAG) // GSZ
                        mov = movb.tile([P, ND, JW], DT_MM, tag="mov")
                        nc.sync.dma_start(
                            out=mov,
                            in_=movp[:, :, mv_idx * JW : (mv_idx + 1) * JW],
                        )
                    ps = psum.tile([P, 2, JW // 2], F32, tag="ps")
                    flat = ps.rearrange("p s j -> p (s j)")
                    for s in range(2):
                        mov_s = mov[:, :, s * (JW // 2) : (s + 1) * (JW // 2)]
                        for dc2 in range(ND // 2):
                            nc.tensor.matmul(
                                ps[:, s, :],
                                statb[:, 2 * dc2 : 2 * dc2 + 2, m * P : (m + 1) * P],
                                mov_s[:, 2 * dc2 : 2 * dc2 + 2, :],
                                start=(dc2 == 0),
                                stop=(dc2 == ND // 2 - 1),
                                perf_mode=mybir.MatmulPerfMode.DoubleRow,
                            )
                    if is_diag:
                        # diag of I-block 8c+u sits at local cols [u*128, ..)
                        off = (m * P) % (JW // 2)
                        nc.vector.tensor_sub(
                            ps[:, (m % NDIAG) // 4, off : off + P],
                            ps[:, (m % NDIAG) // 4, off : off + P],
                            bigI,
                        )
                    ext = expp.tile([P, JW], BF16, tag="ext")
                    idx = m * ns
                    if ns == 2:
                        nc.scalar.activation(
                            out=ext[:, 0:b],
                            in_=flat[:, 0:b],
                            func=mybir.ActivationFunctionType.Exp,
                            scale=SCALE,
                            accum_out=rowacc[:, idx : idx + 1],
                        )
                        nc.scalar.activation(
                            out=ext[:, b:JW],
                            in_=flat[:, b:JW],
                            func=mybir.ActivationFunctionType.Exp,
                            scale=SCALE,
                            accum_out=rowacc[:, idx + 1 : idx + 2],
                        )
                    else:
                        nc.scalar.activation(
                            out=ext,
                            in_=flat,
                            func=mybir.ActivationFunctionType.Exp,
                            scale=SCALE,
                            accum_out=rowacc[:, idx : idx + 1],
                        )
                    if not is_diag:
                        q, r = divmod(m - NDIAG, GSZ)
                        if r == 0:
                            # two 1-bank halves so main psum can triple-buffer
                            mirA = mpsum.tile([2, JW // 2], F32, tag="mirA")
                            mirB = mpsum.tile([2, JW // 2], F32, tag="mirB")
                            mir = (mirA, mirB)
                        for h in range(2):
                            nc.tensor.matmul(
                                mir[h],
                                colw[:, m, :],
                                ext[:, h * (JW // 2) : (h + 1) * (JW // 2)],
                                start=(r == 0),
                                stop=(r == GSZ - 1),
                            )
                        if r == GSZ - 1:
                            mirs = fin.tile([2, JW], F32, tag=f"mirs{q}")
                            nc.vector.tensor_copy(
                                out=mirs[:, 0 : JW // 2], in_=mir[0]
                            )
                            nc.vector.tensor_copy(
                                out=mirs[:, JW // 2 : JW], in_=mir[1]
                            )
                            nc.sync.dma_start(
                                out=mirror_d.ap()[2 * q : 2 * q + 2, :],
                                in_=mirs,
                            )

                nc.sync.dma_start(out=rowacc_d.ap(), in_=rowacc)

    nc.compile()
    _build_cache[key] = nc
    return nc


def make_in_maps(embeddings: np.ndarray, labels: np.ndarray):
    """Returns (in_maps, b, plans, order)."""
    emb = np.asarray(embeddings, dtype=np.float32)
    lab = np.asarray(labels).astype(np.int32)
    order = np.argsort(lab, kind="stable")
    emb_s = emb[order]
    lab_s = lab[order].astype(np.float32)
    n0 = int(np.sum(lab_s == 0))
    b, plans = plan(n0)

    embT = np.ascontiguousarray(emb_s.T)  # [D, N] fp32
    np_dt = mybir.dt.np(DT_MM)
    embT8 = embT.astype(np_dt)
    in_maps = []
    for c in range(NCORES):
        pl = plans[c]
        statp = np.concatenate(
            [embT8[:, t * P : (t + 1) * P] for t, _ in pl["tiles"]], axis=1
        )
        sup_seq = [c] + pl["groups"]
        movp = np.concatenate(
            [embT8[:, g * JW : (g + 1) * JW] for g in sup_seq], axis=1
        )
        colw = np.zeros((P, NT, 2), dtype=np.float32)
        colw[:, :, 0] = 1.0
        for m, (t, _) in enumerate(pl["tiles"]):
            colw[:, m, 1] = lab_s[t * P : (t + 1) * P]
        in_maps.append(
            {
                "statp": np.ascontiguousarray(statp),
                "movp": np.ascontiguousarray(movp),
                "colw": colw.astype(mybir.dt.np(BF16)),
            }
        )
    return in_maps, b, plans, order


def epilogue(outs, labels, b, plans):
    """outs[c] = {'rowacc': [P, NW], 'mirror': [2*NGRP, JW]} (sorted order)."""
    lab = np.asarray(labels).astype(np.int64)
    n0 = int(np.sum(lab == 0))
    lab_srt = np.zeros(N, dtype=np.float64)
    lab_srt[n0:] = 1.0
    ns = 2 if b > 0 else 1
    all_sum = np.zeros(N, dtype=np.float64)
    pos_sum = np.zeros(N, dtype=np.float64)
    for c in range(NCORES):
        ra = np.asarray(outs[c]["rowacc"], np.float64)
        mi = np.asarray(outs[c]["mirror"], np.float64)
        pl = plans[c]
        for m, (t, g) in enumerate(pl["tiles"]):
            rows = slice(t * P, (t + 1) * P)
            if ns == 2:
                a0, a1 = ra[:, 2 * m], ra[:, 2 * m + 1]
                w0 = lab_srt[g * JW]
                w1 = lab_srt[g * JW + b]
                all_sum[rows] += a0 + a1
                pos_sum[rows] += w0 * a0 + w1 * a1
            else:
                a0 = ra[:, m]
                w0 = lab_srt[g * JW]
                all_sum[rows] += a0
                pos_sum[rows] += w0 * a0
        for q, g in enumerate(pl["groups"]):
            rows = slice(g * JW, (g + 1) * JW)
            all_sum[rows] += mi[2 * q, :]
            pos_sum[rows] += mi[2 * q + 1, :]
    loss_rows = np.log(all_sum + EPS) - np.log(pos_sum)
    n_ref = float(lab_srt.sum())
    loss = float(np.sum(loss_rows * lab_srt) / max(n_ref, 1.0))
    return np.float32(loss)


def kernel(embeddings: np.ndarray, labels: np.ndarray) -> np.ndarray:
    lab_f = np.asarray(labels).astype(np.float32)
    n_ref = float(lab_f.sum())
    if n_ref < 2:
        return np.float32(0.0)

    in_maps, b, plans, order = make_in_maps(embeddings, labels)
    nc = build(reps=1, b=b)
    res = run_bass_kernel_spmd(nc, in_maps, core_ids=list(range(NCORES)))
    return epilogue(res.results, labels, b, plans)
